# revision 1
# baseline (speedup 1.0000x reference)
"""Trainium2 Bass kernel for: ConvTranspose2d(128->256, k=4, s=2, p=1)
-> MaxPool2d(2,2) -> Hardtanh -> spatial mean -> Tanh.

Key algebraic restructuring: the stride-2 transposed conv decomposes into 4
polyphase 2x2 convolutions, and the outputs of the 4 phases at pooled
position (i, j) are exactly the 4 elements of the 2x2 maxpool window at
(i, j).  So the 128x128 conv-transpose output is never materialized:

    pooled[b, co, i, j] = max_phase  conv2x2_phase(x)[b, co, i, j] + bias

Everything stays at 64x64 resolution.  Each phase conv is 4 accumulating
K=128 matmuls on the PE array (Cin on partitions, Cout in two 128-halves).
The taps' spatial shifts are realized as AP offsets into a zero-padded
66x66 "canvas" copy of the image in SBUF; the moving operand is a
[8 rows x 64 cols] strided view so only valid output columns are computed
and each 8-row chunk exactly fills one PSUM bank (512 fp32).

Sharding: data-parallel over batch, 8 images per core on 8 cores.
Weights (tiny) replicated.  Matmuls in bf16 (fp32 matmul is 2x slower on
the PE; bf16 keeps ~2e-4 relative error here), accumulation in fp32 PSUM,
phase-max/clip tree in bf16 on DVE, mean+tanh in fp32 on ACT/DVE.
"""

from contextlib import ExitStack

import ml_dtypes
import numpy as np

import concourse.bacc as bacc
import concourse.bass as bass
import concourse.mybir as mybir
import concourse.tile as tile
from concourse.bass_utils import run_bass_kernel_spmd

# Problem dims (hardcoded per contract)
B, CIN, COUT, H, W = 64, 128, 256, 64, 64
NCORES = 8
BPC = B // NCORES  # images per core

WP = 66  # padded row width (1 + 64 + 1)
NROW = 66  # padded rows (1 + 64 + 1)
CVTOT = WP * NROW  # 4356

# Output rows r=1..64 of the canvas grid, 8 chunks x 8 rows; each chunk's
# [8 x 64] valid-column block exactly fills one PSUM bank.
NCHUNK = 8
CHUNK_ROWS = [8] * NCHUNK
CHUNK_R0 = [1 + 8 * i for i in range(NCHUNK)]
GROUPS = [[0, 1, 2, 3], [4, 5, 6, 7]]

F32 = mybir.dt.float32
BF16 = mybir.dt.bfloat16


def _tap(ph: int, a: int):
    """For phase parity ph (0=even output coord, 1=odd) and tap index a,
    return (input shift, kernel index) in one dimension.

    ConvTranspose2d(stride=2, pad=1): out[2q+r] = sum over taps of
    x[q+di] * w[k].  r=0: (di,k) in {(0,1), (-1,3)}; r=1: {(1,0), (0,2)}.
    """
    if ph == 0:
        return (0, 1) if a == 0 else (-1, 3)
    return (1, 0) if a == 0 else (0, 2)


def _wcol(half: int, p: int, t: int) -> int:
    return ((half * 4 + p) * 4 + t) * 128


def build_nc(
    n_imgs: int = BPC,
    n_halves: int = 2,
    groups=None,
    repeat: int = 1,
    fine_psum: bool = False,
    tap_outer: bool = True,
    deep_bufs: bool = False,
) -> bass.Bass:
    """repeat>1 wraps the whole compute in a hardware loop executing it
    `repeat` times — used only for wall-clock timing (amortizes the ~80ms
    axon RPC overhead); the graded path uses repeat=1 (no loop)."""
    if groups is None:
        groups = GROUPS
    nc = bacc.Bacc("TRN2", target_bir_lowering=False, debug=False)

    xc = nc.dram_tensor("xc", [BPC, 128, CVTOT], BF16, kind="ExternalInput")
    wm = nc.dram_tensor("wm", [128, 2 * 4 * 4 * 128], BF16, kind="ExternalInput")
    br = nc.dram_tensor("br", [128, 2], F32, kind="ExternalInput")
    out = nc.dram_tensor("out", [128, 2 * BPC], F32, kind="ExternalOutput")

    Id = mybir.ActivationFunctionType.Identity
    Tanh = mybir.ActivationFunctionType.Tanh
    MAX = mybir.AluOpType.max
    MIN = mybir.AluOpType.min

    with ExitStack() as ctx:
        tc = ctx.enter_context(tile.TileContext(nc))
        consts = ctx.enter_context(tc.tile_pool(name="consts", bufs=1))
        canvp = ctx.enter_context(
            tc.tile_pool(name="canv", bufs=4 if deep_bufs else 3)
        )
        psump = ctx.enter_context(
            tc.tile_pool(name="ps", bufs=4 if fine_psum else 2, space="PSUM")
        )
        evacp = ctx.enter_context(
            tc.tile_pool(name="ev", bufs=12 if deep_bufs else 8)
        )
        mpool = ctx.enter_context(
            tc.tile_pool(name="mt", bufs=4 if deep_bufs else 3)
        )
        accp = ctx.enter_context(
            tc.tile_pool(name="acc", bufs=6 if deep_bufs else 4)
        )

        w_sb = consts.tile([128, 2 * 4 * 4 * 128], BF16, tag="w")
        nc.sync.dma_start(w_sb[:], wm[:, :])
        b_sb = consts.tile([128, 2], F32, tag="b")
        nc.sync.dma_start(b_sb[:], br[:, :])
        s_all = consts.tile([128, 2 * BPC], F32, tag="sums")
        nc.vector.memset(s_all[:], 0.0)
        o_sb = consts.tile([128, 2 * BPC], F32, tag="out")

        def body():
            for img in range(n_imgs):
                canv = canvp.tile([128, CVTOT], BF16, tag="canv")
                nc.sync.dma_start(canv[:], xc[img])
                cv = canv[:].rearrange("p (r c) -> p r c", c=WP)
                for half in range(n_halves):
                    acc = accp.tile([128, len(groups)], F32, tag="acc")
                    for g, chunks in enumerate(groups):
                        nch = len(chunks)
                        evs = []
                        for p in range(4):
                            ph, pw = p >> 1, p & 1
                            if fine_psum:
                                # two 2-bank tiles per phase: finer-grained
                                # WAR release so PE's start-of-phase matmuls
                                # rarely wait on a whole 16-MM evac.
                                psa = psump.tile([128, 2, 512], F32, tag="ps")
                                psb = psump.tile([128, 2, 512], F32, tag="ps")
                                pss = [psa, psb]
                            else:
                                ps4 = psump.tile([128, 4, 512], F32, tag="ps")
                            if tap_outer:
                                mm_iter = [
                                    (ci, ch, t)
                                    for t in range(4)
                                    for ci, ch in enumerate(chunks)
                                ]
                            else:
                                mm_iter = [
                                    (ci, ch, t)
                                    for ci, ch in enumerate(chunks)
                                    for t in range(4)
                                ]
                            for ci, ch, t in mm_iter:
                                r0 = CHUNK_R0[ch]
                                nr = CHUNK_ROWS[ch]
                                dst = (
                                    pss[ci // 2][:, ci % 2, : nr * 64]
                                    if fine_psum
                                    else ps4[:, ci, : nr * 64]
                                )
                                a, bb = t >> 1, t & 1
                                di, _kh = _tap(ph, a)
                                dj, _kw = _tap(pw, bb)
                                col = _wcol(half, p, t)
                                nc.tensor.matmul(
                                    dst,
                                    w_sb[:, col : col + 128],
                                    cv[:, r0 + di : r0 + di + nr, 1 + dj : 65 + dj],
                                    start=(t == 0),
                                    stop=(t == 3),
                                    skip_group_check=tap_outer,
                                )
                            # Evacuate this phase's group (PSUM fp32 -> SBUF
                            # bf16) on ScalarE with the bias add fused in.
                            ev = evacp.tile([128, 4, 512], BF16, tag="ev")
                            if fine_psum:
                                for j in range(2):
                                    nc.scalar.activation(
                                        ev[:, 2 * j : 2 * j + 2, :],
                                        pss[j][:, :, :],
                                        Id,
                                        bias=b_sb[:, half : half + 1],
                                    )
                            else:
                                nc.scalar.activation(
                                    ev[:, :nch, :],
                                    ps4[:, :nch, :],
                                    Id,
                                    bias=b_sb[:, half : half + 1],
                                )
                            evs.append(ev)

                        # max over the 4 phases (= the 2x2 maxpool), then
                        # hardtanh clip, then sum -> one fp32 partial per group.
                        nf = nch * 512
                        m01 = mpool.tile([128, 4 * 512], BF16, tag="m01")
                        nc.vector.tensor_tensor(
                            m01[:, :nf],
                            evs[0][:].rearrange("p a b -> p (a b)")[:, :nf],
                            evs[1][:].rearrange("p a b -> p (a b)")[:, :nf],
                            MAX,
                        )
                        m23 = mpool.tile([128, 4 * 512], BF16, tag="m23")
                        nc.vector.tensor_tensor(
                            m23[:, :nf],
                            evs[2][:].rearrange("p a b -> p (a b)")[:, :nf],
                            evs[3][:].rearrange("p a b -> p (a b)")[:, :nf],
                            MAX,
                        )
                        nc.vector.tensor_tensor(m01[:, :nf], m01[:, :nf], m23[:, :nf], MAX)
                        nc.vector.tensor_scalar(
                            out=m01[:, :nf],
                            in0=m01[:, :nf],
                            scalar1=-1.0,
                            scalar2=1.0,
                            op0=MAX,
                            op1=MIN,
                        )
                        nc.vector.tensor_reduce(
                            acc[:, g : g + 1],
                            m01[:, :nf],
                            axis=mybir.AxisListType.X,
                            op=mybir.AluOpType.add,
                        )
                    idx = img * 2 + half
                    nc.vector.reduce_sum(
                        s_all[:, idx : idx + 1],
                        acc[:, : len(groups)],
                        axis=mybir.AxisListType.X,
                    )


        if repeat > 1:
            with tc.For_i(0, repeat, 1):
                body()
        else:
            body()

        nc.scalar.activation(o_sb[:], s_all[:], Tanh, scale=1.0 / 4096.0)
        nc.sync.dma_start(out[:, :], o_sb[:])

    nc.finalize()
    return nc


_CACHE: dict = {}


def _get_nc() -> bass.Bass:
    if "nc" not in _CACHE:
        _CACHE["nc"] = build_nc()
    return _CACHE["nc"]


def make_in_maps(x: np.ndarray, weight: np.ndarray, bias: np.ndarray):
    x = np.asarray(x, dtype=np.float32)
    weight = np.asarray(weight, dtype=np.float32)
    bias = np.asarray(bias, dtype=np.float32)

    canv = np.zeros((B, 128, CVTOT), dtype=ml_dtypes.bfloat16)
    view = canv.reshape(B, 128, NROW, WP)
    view[:, :, 1:65, 1:65] = x  # cast fp32 -> bf16

    wmv = np.zeros((128, 2 * 4 * 4 * 128), dtype=ml_dtypes.bfloat16)
    for half in range(2):
        for p in range(4):
            ph, pw = p >> 1, p & 1
            for t in range(4):
                a, bb = t >> 1, t & 1
                _di, kh = _tap(ph, a)
                _dj, kw = _tap(pw, bb)
                col = _wcol(half, p, t)
                wmv[:, col : col + 128] = weight[
                    :, half * 128 : (half + 1) * 128, kh, kw
                ]

    brv = np.ascontiguousarray(bias.reshape(2, 128).T, dtype=np.float32)

    return [
        {"xc": canv[c * BPC : (c + 1) * BPC], "wm": wmv, "br": brv}
        for c in range(NCORES)
    ]


def assemble_output(results: list) -> np.ndarray:
    outs = []
    for c in range(NCORES):
        o = np.asarray(results[c]["out"])  # [128, 2*BPC]
        o = o.reshape(128, BPC, 2).transpose(1, 2, 0).reshape(BPC, COUT)
        outs.append(o)
    return np.concatenate(outs, 0).reshape(B, COUT, 1, 1).astype(np.float32)


def kernel(x: np.ndarray, weight: np.ndarray, bias: np.ndarray) -> np.ndarray:
    nc = _get_nc()
    in_maps = make_in_maps(x, weight, bias)
    res = run_bass_kernel_spmd(nc, in_maps, core_ids=list(range(NCORES)))
    return assemble_output(res.results)



# revision 5
# speedup vs baseline: 1.7015x; 1.7015x over previous
"""Trainium2 Bass kernel for: ConvTranspose2d(128->256, k=4, s=2, p=1)
-> MaxPool2d(2,2) -> Hardtanh -> spatial mean -> Tanh.

Algebraic restructuring (same as the bf16 baseline): the stride-2 transposed
conv decomposes into 4 polyphase 2x2 convolutions whose outputs at pooled
position (i, j) are exactly the 4 elements of the 2x2 maxpool window, so the
128x128 intermediate is never materialized and everything stays at 64x64.

v2 speedup: fp8(e4m3) matmuls in DoubleRow perf mode (2 MACs/PE-cell/cycle).
The DoubleRow pair dimension carries the two *column* taps of each phase: the
canvas holds 3 column-shifted copies of the zero-padded image (dj=-1,0,+1) laid
out as adjacent 4224B slabs, so a [128, 2, 512] moving AP with pair-stride 4224
reads both column taps per cycle.  The two *row* taps are the 2 accumulating
matmuls of each PSUM group.  1024 matmuls of N=512 instead of 2048.

Weights are pre-scaled by 32 into fp8 (undone in the final tanh's scale) to
sit in e4m3's normal range; activations stream as raw fp8(x).  PSUM fp32.

Post-processing per 16-row group (2 PSUM banks per phase, 4 phases = 8 banks):
ScalarE evacuates phases 0/1 with the (scaled) bias fused in; VectorE does
max(e0,e1) in bf16 2x mode, max(ps2,ps3) from PSUM, then two fused
scalar_tensor_tensor ops: (t23+b) max m01, and (v max -32) min 32 with the
spatial sum taken by the free accumulator output.  tanh(sum/(32*4096)) at end.

Sharding: data-parallel over batch, 8 images per core on 8 cores.
"""

from contextlib import ExitStack

import ml_dtypes
import numpy as np

import concourse.bacc as bacc
import concourse.bass as bass
import concourse.mybir as mybir
import concourse.tile as tile
from concourse.bass_utils import run_bass_kernel_spmd

# Problem dims (hardcoded per contract)
B, CIN, COUT, H, W = 64, 128, 256, 64, 64
NCORES = 8
BPC = B // NCORES  # images per core

NROW = 66          # padded rows (1 + 64 + 1)
WSLAB = NROW * 64  # 4224 elements per dj-slab
NSLAB = 3          # dj in {-1, 0, +1}
CVTOT = NSLAB * WSLAB

WSCALE = 32.0      # fp8 weight pre-scale; undone in the final tanh

NGRP = 4           # groups of 16 output rows per (img, half)
NCHK = 2           # 8-row chunks per group -> one PSUM bank each

F32 = mybir.dt.float32
BF16 = mybir.dt.bfloat16
FP8 = mybir.dt.float8e4


def _tap(ph: int, a: int):
    """For phase parity ph (0=even output coord, 1=odd) and tap index a,
    return (input shift, kernel index) in one dimension.

    ConvTranspose2d(stride=2, pad=1): out[2q+r] = sum over taps of
    x[q+di] * w[k].  r=0: (di,k) in {(0,1), (-1,3)}; r=1: {(1,0), (0,2)}.
    """
    if ph == 0:
        return (0, 1) if a == 0 else (-1, 3)
    return (1, 0) if a == 0 else (0, 2)


def _kw_pair(pw: int):
    """kw for DoubleRow pair slots (i=0, i=1); i indexes adjacent dj-slabs
    starting at _slab_lo(pw), i.e. i=0 is the smaller dj."""
    return (3, 1) if pw == 0 else (2, 0)


def _slab_lo(pw: int) -> int:
    """First dj-slab of the pair for column parity pw (slab s = dj+1)."""
    return 0 if pw == 0 else 1


def build_nc(n_imgs: int = BPC, repeat: int = 1) -> bass.Bass:
    """repeat>1 wraps the whole compute in a hardware loop executing it
    `repeat` times — used only for wall-clock timing (amortizes the ~80ms
    axon RPC overhead); the graded path uses repeat=1 (no loop)."""
    nc = bacc.Bacc("TRN2", target_bir_lowering=False, debug=False)

    xc = nc.dram_tensor("xc", [BPC, 128, CVTOT], FP8, kind="ExternalInput")
    wm = nc.dram_tensor("wm", [128, 16 * 256], FP8, kind="ExternalInput")
    pbr = nc.dram_tensor("pb", [128, 2], F32, kind="ExternalInput")
    out = nc.dram_tensor("out", [128, 2 * BPC], F32, kind="ExternalOutput")

    Id = mybir.ActivationFunctionType.Identity
    Tanh = mybir.ActivationFunctionType.Tanh
    MAX = mybir.AluOpType.max
    MIN = mybir.AluOpType.min
    ADD = mybir.AluOpType.add
    DR = mybir.MatmulPerfMode.DoubleRow

    with ExitStack() as ctx:
        tc = ctx.enter_context(tile.TileContext(nc))
        consts = ctx.enter_context(tc.tile_pool(name="consts", bufs=1))
        canvp = ctx.enter_context(tc.tile_pool(name="canv", bufs=3))
        psump = ctx.enter_context(tc.tile_pool(name="ps", bufs=4, space="PSUM"))
        evacp = ctx.enter_context(tc.tile_pool(name="ev", bufs=6))
        mpool = ctx.enter_context(tc.tile_pool(name="mt", bufs=8))

        w_sb = consts.tile([128, 16 * 256], FP8, tag="w")
        nc.sync.dma_start(w_sb[:], wm[:, :])
        pb_sb = consts.tile([128, 2], F32, tag="pb")
        nc.sync.dma_start(pb_sb[:], pbr[:, :])
        sums = consts.tile([128, 2 * BPC * NGRP], F32, tag="sums")
        s_red = consts.tile([128, 2 * BPC], F32, tag="sred")
        o_sb = consts.tile([128, 2 * BPC], F32, tag="out")

        nf = NCHK * 512

        def body():
            for img in range(n_imgs):
                canv = canvp.tile([128, CVTOT], FP8, tag="canv")
                nc.sync.dma_start(canv[:], xc[img])
                cv3 = canv[:].rearrange("p (s x) -> p s x", s=NSLAB)
                for half in range(2):
                    pb_ap = pb_sb[:, half : half + 1]
                    for g in range(NGRP):
                        pss = []
                        evs = []
                        for phase in range(4):
                            ph, pw = phase >> 1, phase & 1
                            lo = _slab_lo(pw)
                            ps = psump.tile([128, NCHK, 512], F32, tag="ps")
                            for a in range(2):
                                di, _kh = _tap(ph, a)
                                s = (half * 4 + phase) * 2 + a
                                w_ap = w_sb[
                                    :, s * 256 : (s + 1) * 256
                                ].rearrange("p (i m) -> p i m", i=2)
                                for c in range(NCHK):
                                    r0 = 16 * g + 8 * c
                                    off = (1 + r0 + di) * 64
                                    nc.tensor.matmul(
                                        ps[:, c, :],
                                        w_ap,
                                        cv3[:, lo : lo + 2, off : off + 512],
                                        start=(a == 0),
                                        stop=(a == 1),
                                        perf_mode=DR,
                                        skip_group_check=True,
                                    )
                            pss.append(ps)
                            if phase <= 2:
                                # evacuate phases 0-2 on ScalarE with the
                                # (scaled) bias fused in; phase 3 is consumed
                                # from PSUM by VectorE directly (only one
                                # PSUM operand per DVE instruction allowed).
                                ev = evacp.tile([128, nf], BF16, tag="ev")
                                nc.scalar.activation(
                                    ev[:],
                                    ps[:].rearrange("p a b -> p (a b)"),
                                    Id,
                                    bias=pb_ap,
                                )
                                evs.append(ev)

                        c1 = mpool.tile([128, nf], BF16, tag="c1")
                        nc.vector.tensor_tensor(c1[:], evs[0][:], evs[1][:], MAX)
                        c2 = mpool.tile([128, nf], BF16, tag="c2")
                        nc.vector.scalar_tensor_tensor(
                            c2[:],
                            pss[3][:].rearrange("p a b -> p (a b)"),
                            pb_ap,
                            evs[2][:],
                            ADD,
                            MAX,
                        )
                        u = mpool.tile([128, nf], BF16, tag="u")
                        nc.vector.scalar_tensor_tensor(
                            u[:], c1[:], -WSCALE, c2[:], MAX, MAX
                        )
                        wcl = mpool.tile([128, nf], BF16, tag="wcl")
                        col = (img * 2 + half) * NGRP + g
                        nc.vector.tensor_scalar(
                            wcl[:],
                            u[:],
                            WSCALE,
                            None,
                            MIN,
                            ADD,
                            accum_out=sums[:, col : col + 1],
                        )

        if repeat > 1:
            with tc.For_i(0, repeat, 1):
                body()
        else:
            body()

        nc.vector.tensor_reduce(
            s_red[:],
            sums[:].rearrange("p (i g) -> p i g", g=NGRP),
            axis=mybir.AxisListType.X,
            op=ADD,
        )
        nc.scalar.activation(
            o_sb[:], s_red[:], Tanh, scale=1.0 / (WSCALE * 4096.0)
        )
        nc.sync.dma_start(out[:, :], o_sb[:])

    nc.finalize()
    return nc


_CACHE: dict = {}


def _get_nc() -> bass.Bass:
    if "nc" not in _CACHE:
        _CACHE["nc"] = build_nc()
    return _CACHE["nc"]


def make_in_maps(x: np.ndarray, weight: np.ndarray, bias: np.ndarray):
    x = np.asarray(x, dtype=np.float32)
    weight = np.asarray(weight, dtype=np.float32)
    bias = np.asarray(bias, dtype=np.float32)
    f8 = ml_dtypes.float8_e4m3

    xq = x.astype(f8)  # |x| << 240, no clipping needed
    # 3 column-shifted zero-padded copies: canv[b,s,p,1+r,c] = x[b,p,r,c+dj],
    # slab s = dj+1.
    canv = np.zeros((B, NSLAB, 128, NROW, 64), dtype=f8)
    canv[:, 1, :, 1:65, :] = xq
    canv[:, 0, :, 1:65, 1:64] = xq[:, :, :, 0:63]
    canv[:, 2, :, 1:65, 0:63] = xq[:, :, :, 1:64]
    canvf = np.ascontiguousarray(canv.transpose(0, 2, 1, 3, 4)).reshape(
        B, 128, CVTOT
    )

    wq = np.clip(weight * WSCALE, -240.0, 240.0).astype(f8)  # [cin,cout,kh,kw]
    wmv = np.zeros((128, 16 * 256), dtype=f8)
    for half in range(2):
        blk = wq[:, half * 128 : (half + 1) * 128]  # [128,128,4,4]
        for phase in range(4):
            ph, pw = phase >> 1, phase & 1
            kw0, kw1 = _kw_pair(pw)
            for a in range(2):
                _di, kh = _tap(ph, a)
                s = (half * 4 + phase) * 2 + a
                wmv[:, s * 256 : s * 256 + 128] = blk[:, :, kh, kw0]
                wmv[:, s * 256 + 128 : s * 256 + 256] = blk[:, :, kh, kw1]

    pbv = np.ascontiguousarray(
        (WSCALE * bias).reshape(2, 128).T, dtype=np.float32
    )

    return [
        {"xc": canvf[c * BPC : (c + 1) * BPC], "wm": wmv, "pb": pbv}
        for c in range(NCORES)
    ]


def assemble_output(results: list) -> np.ndarray:
    outs = []
    for c in range(NCORES):
        o = np.asarray(results[c]["out"])  # [128, 2*BPC]
        o = o.reshape(128, BPC, 2).transpose(1, 2, 0).reshape(BPC, COUT)
        outs.append(o)
    return np.concatenate(outs, 0).reshape(B, COUT, 1, 1).astype(np.float32)


def kernel(x: np.ndarray, weight: np.ndarray, bias: np.ndarray) -> np.ndarray:
    nc = _get_nc()
    in_maps = make_in_maps(x, weight, bias)
    res = run_bass_kernel_spmd(nc, in_maps, core_ids=list(range(NCORES)))
    return assemble_output(res.results)


# revision 18
# speedup vs baseline: 1.7555x; 1.0317x over previous
"""Trainium2 Bass kernel for: ConvTranspose2d(128->256, k=4, s=2, p=1)
-> MaxPool2d(2,2) -> Hardtanh -> spatial mean -> Tanh.

Algebraic restructuring (same as the bf16 baseline): the stride-2 transposed
conv decomposes into 4 polyphase 2x2 convolutions whose outputs at pooled
position (i, j) are exactly the 4 elements of the 2x2 maxpool window, so the
128x128 intermediate is never materialized and everything stays at 64x64.

v2 speedup: fp8(e4m3) matmuls in DoubleRow perf mode (2 MACs/PE-cell/cycle).
The DoubleRow pair dimension carries the two *column* taps of each phase: the
canvas holds 3 column-shifted copies of the zero-padded image (dj=-1,0,+1) laid
out as adjacent 4224B slabs, so a [128, 2, 512] moving AP with pair-stride 4224
reads both column taps per cycle.  The two *row* taps are the 2 accumulating
matmuls of each PSUM group.  1024 matmuls of N=512 instead of 2048.

Weights are pre-scaled by 32 into fp8 (undone in the final tanh's scale) to
sit in e4m3's normal range; activations stream as raw fp8(x).  PSUM fp32.

Post-processing per 16-row group (2 PSUM banks per phase, 4 phases = 8 banks):
ScalarE evacuates phases 0/1 with the (scaled) bias fused in; VectorE does
max(e0,e1) in bf16 2x mode, max(ps2,ps3) from PSUM, then two fused
scalar_tensor_tensor ops: (t23+b) max m01, and (v max -32) min 32 with the
spatial sum taken by the free accumulator output.  tanh(sum/(32*4096)) at end.

Sharding: data-parallel over batch, 8 images per core on 8 cores.
"""

import os
from contextlib import ExitStack

import ml_dtypes
import numpy as np

import concourse.bacc as bacc
import concourse.bass as bass
import concourse.mybir as mybir
import concourse.tile as tile
from concourse.bass_utils import run_bass_kernel_spmd

# Problem dims (hardcoded per contract)
B, CIN, COUT, H, W = 64, 128, 256, 64, 64
NCORES = 8
BPC = B // NCORES  # images per core

NROW = 66          # padded rows (1 + 64 + 1)
WSLAB = NROW * 64  # 4224 elements per dj-slab
NSLAB = 3          # dj in {-1, 0, +1}
CVTOT = NSLAB * WSLAB

WSCALE = 32.0      # fp8 weight pre-scale; undone in the final tanh

NCHK = int(os.environ.get("KNCHK", "2"))  # 8-row chunks per group (1 PSUM bank each)
NGRP = 8 // NCHK   # groups of NCHK*8 output rows per (img, half)
NEVAC = int(os.environ.get("KNEVAC", "3"))  # phases evacuated by ScalarE (3 or 4)

F32 = mybir.dt.float32
BF16 = mybir.dt.bfloat16
FP8 = mybir.dt.float8e4


def _tap(ph: int, a: int):
    """For phase parity ph (0=even output coord, 1=odd) and tap index a,
    return (input shift, kernel index) in one dimension.

    ConvTranspose2d(stride=2, pad=1): out[2q+r] = sum over taps of
    x[q+di] * w[k].  r=0: (di,k) in {(0,1), (-1,3)}; r=1: {(1,0), (0,2)}.
    """
    if ph == 0:
        return (0, 1) if a == 0 else (-1, 3)
    return (1, 0) if a == 0 else (0, 2)


def _kw_pair(pw: int):
    """kw for DoubleRow pair slots (i=0, i=1); i indexes adjacent dj-slabs
    starting at _slab_lo(pw), i.e. i=0 is the smaller dj."""
    return (3, 1) if pw == 0 else (2, 0)


def _slab_lo(pw: int) -> int:
    """First dj-slab of the pair for column parity pw (slab s = dj+1)."""
    return 0 if pw == 0 else 1


def build_nc(n_imgs: int = BPC, repeat: int = 1, pe_only: bool | None = None) -> bass.Bass:
    """repeat>1 wraps the whole compute in a hardware loop executing it
    `repeat` times — used only for wall-clock timing (amortizes the ~80ms
    axon RPC overhead); the graded path uses repeat=1 (no loop).

    pe_only drops all post-processing (timing experiment; wrong results)."""
    if pe_only is None:
        pe_only = os.environ.get("KPE_ONLY", "0") == "1"
    nc = bacc.Bacc("TRN2", target_bir_lowering=False, debug=False)

    xc = nc.dram_tensor("xc", [BPC, 128, CVTOT], FP8, kind="ExternalInput")
    wm = nc.dram_tensor("wm", [128, 16 * 256], FP8, kind="ExternalInput")
    pbr = nc.dram_tensor("pb", [128, 2], F32, kind="ExternalInput")
    out = nc.dram_tensor("out", [128, 2 * BPC], F32, kind="ExternalOutput")

    Id = mybir.ActivationFunctionType.Identity
    Tanh = mybir.ActivationFunctionType.Tanh
    MAX = mybir.AluOpType.max
    MIN = mybir.AluOpType.min
    ADD = mybir.AluOpType.add
    DR = mybir.MatmulPerfMode.DoubleRow

    with ExitStack() as ctx:
        tc = ctx.enter_context(tile.TileContext(nc))
        consts = ctx.enter_context(tc.tile_pool(name="consts", bufs=1))
        canvp = ctx.enter_context(tc.tile_pool(name="canv", bufs=3))
        psump = ctx.enter_context(
            tc.tile_pool(name="ps", bufs=8 // NCHK, space="PSUM")
        )
        evacp = ctx.enter_context(tc.tile_pool(name="ev", bufs=8))
        mpool = ctx.enter_context(tc.tile_pool(name="mt", bufs=8))

        w_sb = consts.tile([128, 16 * 256], FP8, tag="w")
        nc.sync.dma_start(w_sb[:], wm[:, :])
        pb_sb = consts.tile([128, 2], F32, tag="pb")
        nc.sync.dma_start(pb_sb[:], pbr[:, :])
        sums = consts.tile([128, 2 * BPC * NGRP], F32, tag="sums")
        nc.vector.memset(sums[:], 0.0)
        s_red = consts.tile([128, 2 * BPC], F32, tag="sred")
        o_sb = consts.tile([128, 2 * BPC], F32, tag="out")

        nf = NCHK * 512

        def body():
            for img in range(n_imgs):
                canv = canvp.tile([128, CVTOT], FP8, tag="canv")
                nc.sync.dma_start(canv[:], xc[img])
                cv3 = canv[:].rearrange("p (s x) -> p s x", s=NSLAB)
                for half in range(2):
                    pb_ap = pb_sb[:, half : half + 1]
                    for g in range(NGRP):
                        pss = []
                        evs = []
                        for phase in range(4):
                            ph, pw = phase >> 1, phase & 1
                            lo = _slab_lo(pw)
                            ps = psump.tile([128, NCHK, 512], F32, tag="ps")
                            for a in range(2):
                                di, _kh = _tap(ph, a)
                                s = (half * 4 + phase) * 2 + a
                                w_ap = w_sb[
                                    :, s * 256 : (s + 1) * 256
                                ].rearrange("p (i m) -> p i m", i=2)
                                for c in range(NCHK):
                                    r0 = 8 * (NCHK * g + c)
                                    off = (1 + r0 + di) * 64
                                    nc.tensor.matmul(
                                        ps[:, c, :],
                                        w_ap,
                                        cv3[:, lo : lo + 2, off : off + 512],
                                        start=(a == 0),
                                        stop=(a == 1),
                                        perf_mode=DR,
                                        skip_group_check=True,
                                    )
                            pss.append(ps)
                            if pe_only:
                                # timing experiment: a 1-element read is the
                                # cheapest consumer that still releases the
                                # PSUM tile for pool reuse.
                                nc.vector.tensor_scalar(
                                    sums[:, :1], ps[:, 0, :1], 0.0, None, ADD
                                )
                                continue
                            if phase < NEVAC:
                                # evacuate phases on ScalarE with the (scaled)
                                # bias fused in; a non-evacuated phase 3 is
                                # consumed from PSUM by VectorE directly (only
                                # one PSUM operand per DVE instruction).
                                ev = evacp.tile([128, nf], BF16, tag="ev")
                                nc.scalar.activation(
                                    ev[:],
                                    ps[:].rearrange("p a b -> p (a b)"),
                                    Id,
                                    bias=pb_ap,
                                )
                                evs.append(ev)

                        if pe_only:
                            continue
                        c1 = mpool.tile([128, nf], BF16, tag="c1")
                        nc.vector.tensor_tensor(c1[:], evs[0][:], evs[1][:], MAX)
                        c2 = mpool.tile([128, nf], BF16, tag="c2")
                        if NEVAC == 4:
                            nc.vector.tensor_tensor(
                                c2[:], evs[2][:], evs[3][:], MAX
                            )
                        else:
                            nc.vector.scalar_tensor_tensor(
                                c2[:],
                                pss[3][:].rearrange("p a b -> p (a b)"),
                                pb_ap,
                                evs[2][:],
                                ADD,
                                MAX,
                            )
                        u = mpool.tile([128, nf], BF16, tag="u")
                        nc.vector.scalar_tensor_tensor(
                            u[:], c1[:], -WSCALE, c2[:], MAX, MAX
                        )
                        wcl = mpool.tile([128, nf], BF16, tag="wcl")
                        col = (img * 2 + half) * NGRP + g
                        nc.vector.tensor_scalar(
                            wcl[:],
                            u[:],
                            WSCALE,
                            None,
                            MIN,
                            ADD,
                            accum_out=sums[:, col : col + 1],
                        )

        if repeat > 1:
            with tc.For_i(0, repeat, 1):
                body()
        else:
            body()

        nc.vector.tensor_reduce(
            s_red[:],
            sums[:].rearrange("p (i g) -> p i g", g=NGRP),
            axis=mybir.AxisListType.X,
            op=ADD,
        )
        nc.scalar.activation(
            o_sb[:], s_red[:], Tanh, scale=1.0 / (WSCALE * 4096.0)
        )
        nc.sync.dma_start(out[:, :], o_sb[:])

    nc.finalize()
    return nc


_CACHE: dict = {}


def _get_nc() -> bass.Bass:
    if "nc" not in _CACHE:
        _CACHE["nc"] = build_nc()
    return _CACHE["nc"]


def make_in_maps(x: np.ndarray, weight: np.ndarray, bias: np.ndarray):
    x = np.asarray(x, dtype=np.float32)
    weight = np.asarray(weight, dtype=np.float32)
    bias = np.asarray(bias, dtype=np.float32)
    f8 = ml_dtypes.float8_e4m3

    xq = x.astype(f8)  # |x| << 240, no clipping needed
    # 3 column-shifted zero-padded copies: canv[b,s,p,1+r,c] = x[b,p,r,c+dj],
    # slab s = dj+1.
    canv = np.zeros((B, NSLAB, 128, NROW, 64), dtype=f8)
    canv[:, 1, :, 1:65, :] = xq
    canv[:, 0, :, 1:65, 1:64] = xq[:, :, :, 0:63]
    canv[:, 2, :, 1:65, 0:63] = xq[:, :, :, 1:64]
    canvf = np.ascontiguousarray(canv.transpose(0, 2, 1, 3, 4)).reshape(
        B, 128, CVTOT
    )

    wq = np.clip(weight * WSCALE, -240.0, 240.0).astype(f8)  # [cin,cout,kh,kw]
    wmv = np.zeros((128, 16 * 256), dtype=f8)
    for half in range(2):
        blk = wq[:, half * 128 : (half + 1) * 128]  # [128,128,4,4]
        for phase in range(4):
            ph, pw = phase >> 1, phase & 1
            kw0, kw1 = _kw_pair(pw)
            for a in range(2):
                _di, kh = _tap(ph, a)
                s = (half * 4 + phase) * 2 + a
                wmv[:, s * 256 : s * 256 + 128] = blk[:, :, kh, kw0]
                wmv[:, s * 256 + 128 : s * 256 + 256] = blk[:, :, kh, kw1]

    pbv = np.ascontiguousarray(
        (WSCALE * bias).reshape(2, 128).T, dtype=np.float32
    )

    return [
        {"xc": canvf[c * BPC : (c + 1) * BPC], "wm": wmv, "pb": pbv}
        for c in range(NCORES)
    ]


def assemble_output(results: list) -> np.ndarray:
    outs = []
    for c in range(NCORES):
        o = np.asarray(results[c]["out"])  # [128, 2*BPC]
        o = o.reshape(128, BPC, 2).transpose(1, 2, 0).reshape(BPC, COUT)
        outs.append(o)
    return np.concatenate(outs, 0).reshape(B, COUT, 1, 1).astype(np.float32)


def kernel(x: np.ndarray, weight: np.ndarray, bias: np.ndarray) -> np.ndarray:
    nc = _get_nc()
    in_maps = make_in_maps(x, weight, bias)
    res = run_bass_kernel_spmd(nc, in_maps, core_ids=list(range(NCORES)))
    return assemble_output(res.results)


# revision 19
# speedup vs baseline: 1.8256x; 1.0400x over previous
"""Trainium2 Bass kernel for: ConvTranspose2d(128->256, k=4, s=2, p=1)
-> MaxPool2d(2,2) -> Hardtanh -> spatial mean -> Tanh.

Algebraic restructuring (same as the bf16 baseline): the stride-2 transposed
conv decomposes into 4 polyphase 2x2 convolutions whose outputs at pooled
position (i, j) are exactly the 4 elements of the 2x2 maxpool window, so the
128x128 intermediate is never materialized and everything stays at 64x64.

v2 speedup: fp8(e4m3) matmuls in DoubleRow perf mode (2 MACs/PE-cell/cycle).
The DoubleRow pair dimension carries the two *column* taps of each phase: the
canvas holds 3 column-shifted copies of the zero-padded image (dj=-1,0,+1) laid
out as adjacent 4224B slabs, so a [128, 2, 512] moving AP with pair-stride 4224
reads both column taps per cycle.  The two *row* taps are the 2 accumulating
matmuls of each PSUM group.  1024 matmuls of N=512 instead of 2048.

Weights are pre-scaled by 32 into fp8 (undone in the final tanh's scale) to
sit in e4m3's normal range; activations stream as raw fp8(x).  PSUM fp32.

Post-processing per 16-row group (2 PSUM banks per phase, 4 phases = 8 banks):
ScalarE evacuates phases 0/1 with the (scaled) bias fused in; VectorE does
max(e0,e1) in bf16 2x mode, max(ps2,ps3) from PSUM, then two fused
scalar_tensor_tensor ops: (t23+b) max m01, and (v max -32) min 32 with the
spatial sum taken by the free accumulator output.  tanh(sum/(32*4096)) at end.

Sharding: data-parallel over batch, 8 images per core on 8 cores.
"""

import os
from contextlib import ExitStack

import ml_dtypes
import numpy as np

import concourse.bacc as bacc
import concourse.bass as bass
import concourse.mybir as mybir
import concourse.tile as tile
from concourse.bass_utils import run_bass_kernel_spmd

# Problem dims (hardcoded per contract)
B, CIN, COUT, H, W = 64, 128, 256, 64, 64
NCORES = 8
BPC = B // NCORES  # images per core

NROW = 66          # padded rows (1 + 64 + 1)
WSLAB = NROW * 64  # 4224 elements per dj-slab
NSLAB = 3          # dj in {-1, 0, +1}
CVTOT = NSLAB * WSLAB

WSCALE = 32.0      # fp8 weight pre-scale; undone in the final tanh

NCHK = int(os.environ.get("KNCHK", "2"))  # 8-row chunks per group (1 PSUM bank each)
NGRP = 8 // NCHK   # groups of NCHK*8 output rows per (img, half)
NEVAC = int(os.environ.get("KNEVAC", "4"))  # phases evacuated by ScalarE (3 or 4)

F32 = mybir.dt.float32
BF16 = mybir.dt.bfloat16
FP8 = mybir.dt.float8e4


def _tap(ph: int, a: int):
    """For phase parity ph (0=even output coord, 1=odd) and tap index a,
    return (input shift, kernel index) in one dimension.

    ConvTranspose2d(stride=2, pad=1): out[2q+r] = sum over taps of
    x[q+di] * w[k].  r=0: (di,k) in {(0,1), (-1,3)}; r=1: {(1,0), (0,2)}.
    """
    if ph == 0:
        return (0, 1) if a == 0 else (-1, 3)
    return (1, 0) if a == 0 else (0, 2)


def _kw_pair(pw: int):
    """kw for DoubleRow pair slots (i=0, i=1); i indexes adjacent dj-slabs
    starting at _slab_lo(pw), i.e. i=0 is the smaller dj."""
    return (3, 1) if pw == 0 else (2, 0)


def _slab_lo(pw: int) -> int:
    """First dj-slab of the pair for column parity pw (slab s = dj+1)."""
    return 0 if pw == 0 else 1


def build_nc(n_imgs: int = BPC, repeat: int = 1, pe_only: bool | None = None) -> bass.Bass:
    """repeat>1 wraps the whole compute in a hardware loop executing it
    `repeat` times — used only for wall-clock timing (amortizes the ~80ms
    axon RPC overhead); the graded path uses repeat=1 (no loop).

    pe_only drops all post-processing (timing experiment; wrong results)."""
    if pe_only is None:
        pe_only = os.environ.get("KPE_ONLY", "0") == "1"
    nc = bacc.Bacc("TRN2", target_bir_lowering=False, debug=False)

    xc = nc.dram_tensor("xc", [BPC, 128, CVTOT], FP8, kind="ExternalInput")
    wm = nc.dram_tensor("wm", [128, 16 * 256], FP8, kind="ExternalInput")
    pbr = nc.dram_tensor("pb", [128, 2], F32, kind="ExternalInput")
    out = nc.dram_tensor("out", [128, 2 * BPC], F32, kind="ExternalOutput")

    Id = mybir.ActivationFunctionType.Identity
    Tanh = mybir.ActivationFunctionType.Tanh
    MAX = mybir.AluOpType.max
    MIN = mybir.AluOpType.min
    ADD = mybir.AluOpType.add
    DR = mybir.MatmulPerfMode.DoubleRow

    with ExitStack() as ctx:
        tc = ctx.enter_context(tile.TileContext(nc))
        consts = ctx.enter_context(tc.tile_pool(name="consts", bufs=1))
        canvp = ctx.enter_context(tc.tile_pool(name="canv", bufs=3))
        psump = ctx.enter_context(
            tc.tile_pool(name="ps", bufs=8 // NCHK, space="PSUM")
        )
        evacp = ctx.enter_context(tc.tile_pool(name="ev", bufs=8))
        mpool = ctx.enter_context(tc.tile_pool(name="mt", bufs=8))

        w_sb = consts.tile([128, 16 * 256], FP8, tag="w")
        nc.sync.dma_start(w_sb[:], wm[:, :])
        pb_sb = consts.tile([128, 2], F32, tag="pb")
        nc.sync.dma_start(pb_sb[:], pbr[:, :])
        sums = consts.tile([128, 2 * BPC * NGRP], F32, tag="sums")
        nc.vector.memset(sums[:], 0.0)
        s_red = consts.tile([128, 2 * BPC], F32, tag="sred")
        o_sb = consts.tile([128, 2 * BPC], F32, tag="out")

        nf = NCHK * 512

        def body():
            for img in range(n_imgs):
                canv = canvp.tile([128, CVTOT], FP8, tag="canv")
                nc.sync.dma_start(canv[:], xc[img])
                cv3 = canv[:].rearrange("p (s x) -> p s x", s=NSLAB)
                for half in range(2):
                    pb_ap = pb_sb[:, half : half + 1]
                    for g in range(NGRP):
                        pss = []
                        evs = []
                        for phase in range(4):
                            ph, pw = phase >> 1, phase & 1
                            lo = _slab_lo(pw)
                            ps = psump.tile([128, NCHK, 512], F32, tag="ps")
                            for a in range(2):
                                di, _kh = _tap(ph, a)
                                s = (half * 4 + phase) * 2 + a
                                w_ap = w_sb[
                                    :, s * 256 : (s + 1) * 256
                                ].rearrange("p (i m) -> p i m", i=2)
                                for c in range(NCHK):
                                    r0 = 8 * (NCHK * g + c)
                                    off = (1 + r0 + di) * 64
                                    nc.tensor.matmul(
                                        ps[:, c, :],
                                        w_ap,
                                        cv3[:, lo : lo + 2, off : off + 512],
                                        start=(a == 0),
                                        stop=(a == 1),
                                        perf_mode=DR,
                                        skip_group_check=True,
                                    )
                            pss.append(ps)
                            if pe_only:
                                # timing experiment: a 1-element read is the
                                # cheapest consumer that still releases the
                                # PSUM tile for pool reuse.
                                nc.vector.tensor_scalar(
                                    sums[:, :1], ps[:, 0, :1], 0.0, None, ADD
                                )
                                continue
                            if phase < NEVAC:
                                # evacuate phases on ScalarE with the (scaled)
                                # bias fused in; a non-evacuated phase 3 is
                                # consumed from PSUM by VectorE directly (only
                                # one PSUM operand per DVE instruction).
                                ev = evacp.tile([128, nf], BF16, tag="ev")
                                nc.scalar.activation(
                                    ev[:],
                                    ps[:].rearrange("p a b -> p (a b)"),
                                    Id,
                                    bias=pb_ap,
                                )
                                evs.append(ev)

                        if pe_only:
                            continue
                        c1 = mpool.tile([128, nf], BF16, tag="c1")
                        nc.vector.tensor_tensor(c1[:], evs[0][:], evs[1][:], MAX)
                        c2 = mpool.tile([128, nf], BF16, tag="c2")
                        if NEVAC == 4:
                            nc.vector.tensor_tensor(
                                c2[:], evs[2][:], evs[3][:], MAX
                            )
                        else:
                            nc.vector.scalar_tensor_tensor(
                                c2[:],
                                pss[3][:].rearrange("p a b -> p (a b)"),
                                pb_ap,
                                evs[2][:],
                                ADD,
                                MAX,
                            )
                        u = mpool.tile([128, nf], BF16, tag="u")
                        nc.vector.scalar_tensor_tensor(
                            u[:], c1[:], -WSCALE, c2[:], MAX, MAX
                        )
                        wcl = mpool.tile([128, nf], BF16, tag="wcl")
                        col = (img * 2 + half) * NGRP + g
                        nc.vector.tensor_scalar(
                            wcl[:],
                            u[:],
                            WSCALE,
                            None,
                            MIN,
                            ADD,
                            accum_out=sums[:, col : col + 1],
                        )

        if repeat > 1:
            with tc.For_i(0, repeat, 1):
                body()
        else:
            body()

        nc.vector.tensor_reduce(
            s_red[:],
            sums[:].rearrange("p (i g) -> p i g", g=NGRP),
            axis=mybir.AxisListType.X,
            op=ADD,
        )
        nc.scalar.activation(
            o_sb[:], s_red[:], Tanh, scale=1.0 / (WSCALE * 4096.0)
        )
        nc.sync.dma_start(out[:, :], o_sb[:])

    nc.finalize()
    return nc


_CACHE: dict = {}


def _get_nc() -> bass.Bass:
    if "nc" not in _CACHE:
        _CACHE["nc"] = build_nc()
    return _CACHE["nc"]


def make_in_maps(x: np.ndarray, weight: np.ndarray, bias: np.ndarray):
    x = np.asarray(x, dtype=np.float32)
    weight = np.asarray(weight, dtype=np.float32)
    bias = np.asarray(bias, dtype=np.float32)
    f8 = ml_dtypes.float8_e4m3

    xq = x.astype(f8)  # |x| << 240, no clipping needed
    # 3 column-shifted zero-padded copies: canv[b,s,p,1+r,c] = x[b,p,r,c+dj],
    # slab s = dj+1.
    canv = np.zeros((B, NSLAB, 128, NROW, 64), dtype=f8)
    canv[:, 1, :, 1:65, :] = xq
    canv[:, 0, :, 1:65, 1:64] = xq[:, :, :, 0:63]
    canv[:, 2, :, 1:65, 0:63] = xq[:, :, :, 1:64]
    canvf = np.ascontiguousarray(canv.transpose(0, 2, 1, 3, 4)).reshape(
        B, 128, CVTOT
    )

    wq = np.clip(weight * WSCALE, -240.0, 240.0).astype(f8)  # [cin,cout,kh,kw]
    wmv = np.zeros((128, 16 * 256), dtype=f8)
    for half in range(2):
        blk = wq[:, half * 128 : (half + 1) * 128]  # [128,128,4,4]
        for phase in range(4):
            ph, pw = phase >> 1, phase & 1
            kw0, kw1 = _kw_pair(pw)
            for a in range(2):
                _di, kh = _tap(ph, a)
                s = (half * 4 + phase) * 2 + a
                wmv[:, s * 256 : s * 256 + 128] = blk[:, :, kh, kw0]
                wmv[:, s * 256 + 128 : s * 256 + 256] = blk[:, :, kh, kw1]

    pbv = np.ascontiguousarray(
        (WSCALE * bias).reshape(2, 128).T, dtype=np.float32
    )

    return [
        {"xc": canvf[c * BPC : (c + 1) * BPC], "wm": wmv, "pb": pbv}
        for c in range(NCORES)
    ]


def assemble_output(results: list) -> np.ndarray:
    outs = []
    for c in range(NCORES):
        o = np.asarray(results[c]["out"])  # [128, 2*BPC]
        o = o.reshape(128, BPC, 2).transpose(1, 2, 0).reshape(BPC, COUT)
        outs.append(o)
    return np.concatenate(outs, 0).reshape(B, COUT, 1, 1).astype(np.float32)


def kernel(x: np.ndarray, weight: np.ndarray, bias: np.ndarray) -> np.ndarray:
    nc = _get_nc()
    in_maps = make_in_maps(x, weight, bias)
    res = run_bass_kernel_spmd(nc, in_maps, core_ids=list(range(NCORES)))
    return assemble_output(res.results)
